# revision 16
# baseline (speedup 1.0000x reference)
"""Trainium2 Bass kernel for nn_MultiHeadAttention (B=8, S=1024, D=768, H=12).

Sharding: data-parallel over batch — one batch element per NeuronCore (8 cores).
No collectives needed; gather is a host-side stack.

bf16 compute with fp32 PSUM accumulation. Per-core layout:
  inputs (host-prepped, bf16): xqT/xkT/xvT (D,S); WqT/WkT (D,D); WvT_pad
  (D, 12*65) with zero columns at each head's slot 64; WoT (D,D); I128
  identity; fp32 biases (bv folded into bo on host: bo_eff = bo + Wo @ bv).
  - QT[do,s] = WqT.T @ xqT + bq ; KT[do,s] = WkT.T @ xkT + bk  (feature-major)
  - V[t,dpad] = xvT.T @ WvT_pad (natural layout, 65-wide head slots with a
    ones column per head so attn@V also yields the softmax denominator)
  - per head pair j (heads 2j at partitions 0:64, 2j+1 at 64:128):
      scoresT[t,s] = KT_h.T @ QT_h   (row-packed K=64 matmul pair)
      E = exp(SCALE * scoresT)       (ScalarE, PSUM->SBUF bf16, both heads)
  - attn@V runs with E stationary (M=128 queries) and V moving (N=65):
      O_sd[s, d+Z] += E_tile.T @ V_aug   — 65-cycle matmuls, fp32 PSUM
    normalize per partition (Z is a column): O_sd[:,0:64] *= 1/Z, then
    PE-transpose the assembled [s,128] pair tile back to feature-major OHT.
  - O[s,do] = OHT.T @ WoT + bo_eff
"""
import sys

sys.path.insert(0, "/opt/trn_rl_repo")

import numpy as np
from ml_dtypes import bfloat16

import concourse.bacc as bacc
import concourse.tile as tile
from concourse import mybir
from concourse.bass_utils import run_bass_kernel_spmd

B, S, D, H = 8, 1024, 768, 12
DH = D // H                       # 64
NP = H // 2                       # 6 head pairs == D/128 tiles
DVP = H * (DH + 1)                # 780: V padded width (65 per head)
SCALE = 1.0 / np.sqrt(np.float32(D))
NT = S // 128                     # 8 seq tiles of 128
ND = D // 128                     # 6 feature tiles of 128

F32 = mybir.dt.float32
BF16 = mybir.dt.bfloat16
Exp = mybir.ActivationFunctionType.Exp

_CACHE = {}


def _build_nc(debug_outputs=False, loop_n=1):
    nc = bacc.Bacc("TRN2", target_bir_lowering=False, debug=False)

    d = {}
    for name, shape in [
        ("xqt", (D, S)), ("xkt", (D, S)), ("xvt", (D, S)),
        ("wqt", (D, D)), ("wkt", (D, D)), ("wvtp", (D, DVP)), ("wot", (D, D)),
        ("ident", (128, 128)),
    ]:
        d[name] = nc.dram_tensor(name, shape, BF16, kind="ExternalInput").ap()
    for name, shape in [("bqc", (128, ND)), ("bkc", (128, ND)),
                        ("bor", (1, D))]:
        d[name] = nc.dram_tensor(name, shape, F32, kind="ExternalInput").ap()
    out_d = nc.dram_tensor("out", (S, D), F32, kind="ExternalOutput").ap()

    with tile.TileContext(nc) as tc:
        for _ in range(loop_n):
            _emit(nc, tc, d, out_d)
    nc.compile()
    return nc


def _emit(nc, tc, d, out_d):
    import contextlib

    ctx = contextlib.ExitStack()
    with ctx:
        w_pool = ctx.enter_context(tc.tile_pool(name="w", bufs=24))
        x_pool = ctx.enter_context(tc.tile_pool(name="x", bufs=18))
        qk_pool = ctx.enter_context(tc.tile_pool(name="qk", bufs=12))
        v_pool = ctx.enter_context(tc.tile_pool(name="v", bufs=8))
        e_pool = ctx.enter_context(tc.tile_pool(name="e", bufs=5))
        oht_pool = ctx.enter_context(tc.tile_pool(name="oht", bufs=6))
        o_pool = ctx.enter_context(tc.tile_pool(name="o", bufs=2))
        osd_pool = ctx.enter_context(tc.tile_pool(name="osd", bufs=6))
        rz_pool = ctx.enter_context(tc.tile_pool(name="rz", bufs=4))
        const_pool = ctx.enter_context(tc.tile_pool(name="const", bufs=1))
        ps = ctx.enter_context(tc.tile_pool(name="ps", bufs=2, space="PSUM"))
        ps_acc = ctx.enter_context(
            tc.tile_pool(name="ps_acc", bufs=4, space="PSUM"))

        # ---- constants ----
        bq_t = const_pool.tile([128, ND], F32, name="bq_t")
        bk_t = const_pool.tile([128, ND], F32, name="bk_t")
        bo_bc = const_pool.tile([128, D], F32, name="bo_bc")
        ident = const_pool.tile([128, 128], BF16, name="ident")
        nc.gpsimd.dma_start(bq_t[:], d["bqc"][:])
        nc.gpsimd.dma_start(bk_t[:], d["bkc"][:])
        nc.gpsimd.dma_start(bo_bc[:], d["bor"].to_broadcast((128, D)))
        nc.gpsimd.dma_start(ident[:], d["ident"][:])

        def load_wx(wkey, wwidth, xkey):
            wt, xt = [], []
            for i in range(ND):
                w = w_pool.tile([128, wwidth], BF16, name=f"{wkey}{i}",
                                tag="w")
                nc.sync.dma_start(w[:], d[wkey][i * 128:(i + 1) * 128, :])
                x = x_pool.tile([128, S], BF16, name=f"{xkey}{i}", tag="x")
                eng = nc.scalar
                eng.dma_start(x[:], d[xkey][i * 128:(i + 1) * 128, :])
                wt.append(w)
                xt.append(x)
            return wt, xt

        def proj_qk(which, w_t, x_t, b_t, p):
            # Q/K projections share the sc (ps) pool; the 2-slot rotation
            # serializes them against in-flight score tiles, which is the
            # intended filler behaviour
            ps_p = ps.tile([128, S], F32, name=f"{which}ps{p}", tag="ps")
            halves = (ps_p[:, 0:512], ps_p[:, 512:S])
            for di in range(ND):
                st, sp = di == 0, di == ND - 1
                lhs = w_t[di][:, p * 128:(p + 1) * 128]
                nc.tensor.matmul(halves[0], lhs, x_t[di][:, 0:512],
                                 start=st, stop=sp)
                nc.tensor.matmul(halves[1], lhs, x_t[di][:, 512:S],
                                 start=st, stop=sp)
            ot = qk_pool.tile([128, S], BF16, name=f"{which}t{p}", tag="qk")
            nc.vector.tensor_scalar_add(ot[:, 0:512], halves[0],
                                        b_t[:, p:p + 1])
            nc.vector.tensor_scalar_add(ot[:, 512:S], halves[1],
                                        b_t[:, p:p + 1])
            return ot

        # ---- V projection first (Q/K weight+activation DMAs queued too,
        # so their projections can interleave with early attention) ----
        wv, xv = load_wx("wvtp", DVP, "xvt")
        wq, xq = load_wx("wqt", D, "xqt")
        wk, xk = load_wx("wkt", D, "xkt")
        v_tiles = []
        for tb in range(NT):
            pa = ps_acc.tile([128, 512], F32, name=f"vpsa{tb}", tag="acc")
            pb = ps_acc.tile([128, DVP - 512], F32, name=f"vpsb{tb}",
                             tag="acc")
            for di in range(ND):
                st, sp = di == 0, di == ND - 1
                lhs = xv[di][:, tb * 128:(tb + 1) * 128]
                nc.tensor.matmul(pa[:], lhs, wv[di][:, 0:512],
                                 start=st, stop=sp)
                nc.tensor.matmul(pb[:], lhs, wv[di][:, 512:DVP],
                                 start=st, stop=sp)
            vt = v_pool.tile([128, DVP], BF16, name=f"v{tb}", tag="v")
            nc.vector.tensor_copy(vt[:, 0:512], pa[:])
            nc.vector.tensor_copy(vt[:, 512:DVP], pb[:])
            # ones columns (head slot 64) for the denominator trick
            v3 = vt[:].rearrange("p (h e) -> p h e", e=DH + 1)
            nc.vector.memset(v3[:, :, DH:DH + 1], 1.0)
            v_tiles.append(vt)

        # ---- first Q/K pair only; the rest become pipeline filler ----
        qt_tiles = [proj_qk("q", wq, xq, bq_t, 0)]
        kt_tiles = [proj_qk("k", wk, xk, bk_t, 0)]

        # ---- attention + output projection ----
        oht_tiles = [
            oht_pool.tile([128, S], BF16, name=f"oht{p}", tag="oht")
            for p in range(NP)
        ]

        def o_proj(stt):
            # one 2-bank slot from the sc pool holds both output halves
            op = ps.tile([128, S], F32, name=f"ops{stt}", tag="ps")
            pa, pb = op[:, 0:512], op[:, 512:768]
            ssl = slice(stt * 128, (stt + 1) * 128)
            for di in range(ND):
                nc.tensor.matmul(pa, oht_tiles[di][:, ssl],
                                 wo[di][:, 0:512],
                                 start=di == 0, stop=di == ND - 1)
                nc.tensor.matmul(pb, oht_tiles[di][:, ssl],
                                 wo[di][:, 512:768],
                                 start=di == 0, stop=di == ND - 1)
            o_t = o_pool.tile([128, D], F32, name=f"o{stt}", tag="o")
            nc.vector.tensor_add(o_t[:, 0:512], pa, bo_bc[:, 0:512])
            nc.sync.dma_start(out_d[ssl, 0:512], o_t[:, 0:512])
            nc.vector.tensor_add(o_t[:, 512:768], pb, bo_bc[:, 512:768])
            nc.sync.dma_start(out_d[ssl, 512:768], o_t[:, 512:768])

        # ---- software-pipelined attention ----
        # Per (pair,strip) the stages are: scores+exp per key tile (tb),
        # attn@V d-stream matmuls lagging 2 tb behind, then normalize +
        # transpose. The last two d-MM batches and the finisher of pair i
        # are deferred into pair i+1's slot so the PE always has filler
        # work while the tail exps drain on ScalarE.
        st_ctx = {}

        def att_state(p, strip):
            return st_ctx.setdefault((p, strip), {"et": {}, "grp": None,
                                                  "osd": None})

        def att_step(p, strip, tb):
            s = att_state(p, strip)
            sl = slice(strip * 512, strip * 512 + 512)
            sc = ps.tile([128, 1024], F32, name=f"sc{p}_{strip}_{tb}",
                         tag="ps")
            tsl = slice(tb * 128, (tb + 1) * 128)
            nc.tensor.matmul(sc[:, 0:512], kt_tiles[p][0:64, tsl],
                             qt_tiles[p][0:64, sl], start=True, stop=True)
            nc.tensor.matmul(sc[:, 512:1024], kt_tiles[p][64:128, tsl],
                             qt_tiles[p][64:128, sl], start=True, stop=True)
            et = e_pool.tile([128, 1024], BF16, name=f"e{p}_{strip}_{tb}",
                             tag="e")
            nc.scalar.activation(et[:], sc[:], Exp, scale=float(SCALE))
            s["et"][tb] = et

        def att_dmm(p, strip, tb):
            s = att_state(p, strip)
            if s["grp"] is None:
                s["grp"] = [ps_acc.tile([128, 260], F32,
                                        name=f"g{h}_{p}_{strip}", tag="acc")
                            for h in (0, 1)]
            c0 = p * 2 * (DH + 1)
            et = s["et"].pop(tb)
            vt = v_tiles[tb]
            for h in (0, 1):
                vsl = vt[:, c0 + h * (DH + 1):c0 + (h + 1) * (DH + 1)]
                for j in range(4):
                    # start=True clears has_written for the WHOLE bank: only
                    # the bank's first matmul may set it; the other groups'
                    # first writes overwrite via cleared bits
                    nc.tensor.matmul(
                        s["grp"][h][:, j * 65:(j + 1) * 65],
                        et[:, h * 512 + j * 128:h * 512 + (j + 1) * 128],
                        vsl, start=tb == 0 and j == 0,
                        stop=tb == NT - 1 and j == 3,
                        skip_group_check=True)

        def att_norm(p, strip):
            # normalize (Z is column 64 of each group); per-partition scalars,
            # all 4 query-subtile groups of a head in one strided DVE op
            s = att_state(p, strip)
            osd = osd_pool.tile([128, 512], BF16, name=f"osd{p}{strip}",
                                tag="osd")
            osd3 = osd[:].rearrange("p (j q) -> p j q", q=128)
            for h in (0, 1):
                rz = rz_pool.tile([128, 4], F32, name=f"rz{p}{strip}{h}",
                                  tag="rz")
                nc.vector.reciprocal_approx_fast(
                    rz[:], s["grp"][h][:, DH::DH + 1])
                g3 = s["grp"][h][:].rearrange("p (j e) -> p j e", e=DH + 1)
                nc.vector.tensor_mul(
                    osd3[:, :, h * DH:(h + 1) * DH], g3[:, :, 0:DH],
                    rz[:].unsqueeze(-1).broadcast_to((128, 4, DH)))
            s["osd"] = osd

        def att_transpose(p, strip):
            s = st_ctx.pop((p, strip))
            sl = slice(strip * 512, strip * 512 + 512)
            tr = ps_acc.tile([128, 512], BF16, name=f"tr{p}_{strip}",
                             tag="acc")
            for j in range(4):
                nc.tensor.transpose(tr[:, j * 128:(j + 1) * 128],
                                    s["osd"][:, j * 128:(j + 1) * 128],
                                    ident[:])
            nc.vector.tensor_copy(oht_tiles[p][:, sl], tr[:])

        wo = []

        def filler(p, strip):
            # PE work with no dependency on the in-flight exps
            if strip == 0:
                if p + 1 < NP:
                    qt_tiles.append(proj_qk("q", wq, xq, bq_t, p + 1))
                    kt_tiles.append(proj_qk("k", wk, xk, bk_t, p + 1))
                else:
                    for i in range(ND):
                        t = w_pool.tile([128, D], BF16, name=f"wot{i}",
                                        tag="w")
                        nc.sync.dma_start(
                            t[:], d["wot"][i * 128:(i + 1) * 128, :])
                        wo.append(t)
            else:
                if 1 <= p <= 4:
                    o_proj(p - 1)

        seq = [(p, 0) for p in range(NP)] + [(p, 1) for p in range(NP)]
        pend = None
        for p, strip in seq:
            filler(p, strip)
            att_step(p, strip, 0)
            att_step(p, strip, 1)
            if pend is not None:
                att_dmm(*pend, NT - 2)
                att_dmm(*pend, NT - 1)
                att_norm(*pend)
            for tb in range(2, NT):
                att_step(p, strip, tb)
                att_dmm(p, strip, tb - 2)
                if tb == 3 and pend is not None:
                    # transposes held back so they never wait on the
                    # normalize chain still draining on DVE
                    att_transpose(*pend)
            pend = (p, strip)
        att_dmm(*pend, NT - 2)
        att_dmm(*pend, NT - 1)
        att_norm(*pend)
        att_transpose(*pend)
        for stt in range(4, 8):
            o_proj(stt)


def _prep(queries, keys, values, Wq, bq, Wk, bk, Wv, bv, Wo, bo):
    """Host-side prep: returns per-core input dicts."""
    wvt = np.asarray(Wv, np.float32).T              # (D, D) = (di, do)
    wvtp = np.zeros((D, DVP), np.float32)
    for h in range(H):
        wvtp[:, h * (DH + 1):h * (DH + 1) + DH] = \
            wvt[:, h * DH:(h + 1) * DH]
    bo_eff = (np.asarray(bo, np.float32)
              + np.asarray(Wo, np.float32) @ np.asarray(bv, np.float32))
    bf = lambda a: np.ascontiguousarray(np.asarray(a, np.float32)).astype(
        bfloat16)
    shared = {
        "wqt": bf(np.asarray(Wq, np.float32).T),
        "wkt": bf(np.asarray(Wk, np.float32).T),
        "wvtp": wvtp.astype(bfloat16),
        "wot": bf(np.asarray(Wo, np.float32).T),
        "ident": np.eye(128, dtype=np.float32).astype(bfloat16),
        "bqc": np.ascontiguousarray(
            np.asarray(bq, np.float32).reshape(ND, 128).T),
        "bkc": np.ascontiguousarray(
            np.asarray(bk, np.float32).reshape(ND, 128).T),
        "bor": np.ascontiguousarray(bo_eff.reshape(1, D)),
    }
    queries = np.asarray(queries, np.float32)
    keys = np.asarray(keys, np.float32)
    values = np.asarray(values, np.float32)
    in_maps = []
    for b in range(B):
        in_maps.append({
            "xqt": bf(queries[b].T),
            "xkt": bf(keys[b].T),
            "xvt": bf(values[b].T),
            **shared,
        })
    return in_maps


def _get_nc():
    if "nc" not in _CACHE:
        _CACHE["nc"] = _build_nc()
    return _CACHE["nc"]


def kernel(queries, keys, values, Wq, bq, Wk, bk, Wv, bv, Wo, bo):
    in_maps = _prep(queries, keys, values, Wq, bq, Wk, bk, Wv, bv, Wo, bo)
    nc = _get_nc()
    res = run_bass_kernel_spmd(nc, in_maps, core_ids=list(range(B)))
    return np.stack([res.results[b]["out"] for b in range(B)], axis=0)
